# revision 18
# baseline (speedup 1.0000x reference)
"""MoE FFN (8 experts, top-2) Trainium2 kernel.

Strategy (expert-parallel, per sharding hint):
  - Host computes the gate (softmax + top-2 + renormalize) in float64 and
    routes tokens: core e receives the tokens whose top-2 includes expert e,
    padded to a common N_pad, transposed to [d_model, N_pad].
  - Each core runs the dense expert FFN for its expert:
        H^T = silu(W1^T x^T) * (W3^T x^T)
        y^T = W2^T H^T
    in one Bass/Tile program; H^T is staged through DRAM in bf16.
  - Host gathers y per expert and combines with the gate weights.

All weights are host-retiled so that every SBUF-bound DMA reads one fully
contiguous DRAM block (the DMA engines run ~8x slower on 512B-row strided
patterns). H^T is staged chunk-major (one DRAM buffer per n-chunk) so both
its writes and reads are contiguous. DMA traffic is spread across the
sync/scalar/gpsimd/vector queues so weight, x, h and w2 streams never queue
behind each other. y^T is returned in bf16 (the +0.2% quantization is well
inside the error budget) to halve the drain tail.

silu(g) is computed as g*sigmoid(g) (ACT sigmoid + DVE muls).
All matmul operands are bf16 (fp32 accumulation in PSUM): bf16 weight
loads get FWL (2x) so LDWEIGHTS hides fully under the matmuls.
"""

import math
from contextlib import ExitStack

import ml_dtypes
import numpy as np

P = 128
D_MODEL = 2048
HIDDEN = 5632
N_EXPERTS = 8
TOP_K = 2
N_CORES = 8

H_SPAN = 256    # phase-1 weight streaming span along hidden dim
D_SPAN = 512    # phase-2 resident W2 span along d_model
DT_SWEEP = 4    # d-tiles accumulated concurrently in phase 2
WARM_MM = 72    # PE warm-up matmuls (pstate ramp + HAM un-throttle)
WARM_W = 512    # warm-up matmul width (sized to span the startup DMA window)
N_DIRECT = 2    # n-chunks whose H^T strips are written directly in SBUF

_prog_cache: dict[int, object] = {}


def _chunk_list(n_pad: int, max_chunk: int = 512) -> list[tuple[int, int]]:
    """Split [0, n_pad) into near-equal chunks <= max_chunk, multiples of 8."""
    assert n_pad % 4 == 0
    k = math.ceil(n_pad / max_chunk)
    base = (n_pad // k) // 4 * 4
    sizes = [base] * k
    extra = n_pad - base * k
    i = 0
    while extra > 0:
        sizes[i] += 4
        extra -= 4
        i = (i + 1) % k
    out = []
    n0 = 0
    for s in sizes:
        out.append((n0, s))
        n0 += s
    return out


def _build_program(d_model: int, hidden: int, n_pad: int):
    import concourse.bacc as bacc
    import concourse.mybir as mybir
    import concourse.tile as tile

    f32 = mybir.dt.float32
    bf16 = mybir.dt.bfloat16
    Sigmoid = mybir.ActivationFunctionType.Sigmoid
    mult = mybir.AluOpType.mult

    DC = d_model // P      # d chunks (contraction tiles for phase 1)
    HC = hidden // P       # h chunks
    HG = hidden // H_SPAN  # phase-1 weight groups
    HL = H_SPAN // P
    DG = d_model // D_SPAN
    DTS = D_SPAN // P
    assert DTS == DT_SWEEP
    chunks = _chunk_list(n_pad)
    NCH = len(chunks)
    ND = min(N_DIRECT, NCH)

    nc = bacc.Bacc(
        "TRN2",
        target_bir_lowering=False,
        debug=False,
        enable_asserts=False,
        num_devices=N_CORES,
    )
    xT = nc.dram_tensor("xT", [d_model, n_pad], bf16, kind="ExternalInput").ap()
    # w1t/w3t rows g*P+p, cols c*H_SPAN+j  <-  W[c*P+p, g*H_SPAN+j]
    w1t = nc.dram_tensor("w1t", [HG * P, DC * H_SPAN], bf16, kind="ExternalInput").ap()
    w3t = nc.dram_tensor("w3t", [HG * P, DC * H_SPAN], bf16, kind="ExternalInput").ap()
    # w2t rows (dg*HC+h)*P+p, cols j  <-  W2[h*P+p, dg*D_SPAN+j]
    w2t = nc.dram_tensor("w2t", [DG * HC * P, D_SPAN], bf16, kind="ExternalInput").ap()
    # chunk-major H^T staging, only for the non-direct chunks
    hbs = {
        j: nc.dram_tensor(f"hb{j}", [HC * P, sz], bf16).ap()
        for j, (n0, sz) in enumerate(chunks)
        if j >= ND
    }
    yts = [
        nc.dram_tensor(f"yt{j}", [d_model, sz], bf16, kind="ExternalOutput").ap()
        for j, (n0, sz) in enumerate(chunks)
    ]
    warm_sink = hbs[ND] if ND < NCH else None

    with tile.TileContext(nc) as tc, ExitStack() as ctx:
        # pools opened before phase 1: their SBUF ranges coexist with the
        # phase-1 pools. W2's dg0 tiles and the direct H^T strips are filled
        # during phase 1 so phase 2 can start with zero transition stall.
        w2pool = ctx.enter_context(tc.tile_pool(name="w2p", bufs=1))
        hspool = ctx.enter_context(tc.tile_pool(name="hsd", bufs=1))
        hstrips = {}
        for j in range(ND):
            sz = chunks[j][1]
            hstrips[j] = hspool.tile([P, HC * sz], bf16, tag=f"hs{j}", name=f"hs{j}")

        # ---- phase 1: H^T = silu(W1^T x^T) * (W3^T x^T)
        with ExitStack() as p1:
            wpool = p1.enter_context(tc.tile_pool(name="w13", bufs=2))
            # first weight group's DMAs go ahead of everything else on their
            # queues so real compute can start ~6us in
            w1g0 = wpool.tile([P, DC * H_SPAN], bf16, tag="w1g", name="w1g0")
            w3g0 = wpool.tile([P, DC * H_SPAN], bf16, tag="w3g", name="w3g0")
            nc.sync.dma_start(out=w1g0[:], in_=w1t[0:P, :])
            nc.scalar.dma_start(out=w3g0[:], in_=w3t[0:P, :])

            # resident x^T, one tile per 128-row d-chunk (fine-grained deps);
            # split across all three DMA queues
            xpool = p1.enter_context(tc.tile_pool(name="xp", bufs=1))
            xts = []
            for c in range(DC):
                t = xpool.tile([P, n_pad], bf16, tag=f"x{c}", name=f"x{c}")
                eng = (nc.gpsimd, nc.sync, nc.scalar)[c % 3]
                eng.dma_start(out=t[:], in_=xT[c * P : (c + 1) * P, :])
                xts.append(t)

            def xslice(c, n0, sz):
                return xts[c][:, n0 : n0 + sz]

            pspool = p1.enter_context(tc.tile_pool(name="ps1", bufs=1, space="PSUM"))
            spool = p1.enter_context(tc.tile_pool(name="sg", bufs=3))
            hpool = p1.enter_context(tc.tile_pool(name="hout", bufs=3))

            # PE warm-up on a memset tile during the initial DMA wait, so the
            # pstate/HAM ramp happens on junk work. The result lands in the
            # DRAM h-staging buffer (fully overwritten later; same-queue WAW
            # keeps it ordered and live).
            wsrc = spool.tile([P, P], bf16, tag="warm_src", name="wsrc")
            nc.vector.memset(wsrc[:], 0.0)
            wsrc2 = spool.tile([P, WARM_W], bf16, tag="warm_src2", name="wsrc2")
            nc.vector.memset(wsrc2[:], 0.0)
            wps = pspool.tile([P, WARM_W], f32, tag="warm", name="warm_ps")
            for i in range(WARM_MM):
                nc.tensor.matmul(
                    wps[:], wsrc[:], wsrc2[:],
                    start=(i == 0), stop=(i == WARM_MM - 1),
                )
            wsb = spool.tile([P, WARM_W], bf16, tag="warm_sb", name="wsb")
            nc.scalar.copy(wsb[:], wps[:])
            if warm_sink is not None:
                sink_w = min(WARM_W, chunks[ND][1])
                nc.gpsimd.dma_start(
                    out=warm_sink[0:P, 0:sink_w], in_=wsb[:, :sink_w]
                )

            # schedule dg0's W2 tile loads into the sync queue behind w1
            # groups 8..21 so W2 is resident well before phase 2 without
            # delaying the w1 stream (wpool bufs=2 absorbs the inserts)
            w2g0_tiles = [None] * HC
            w2_sched = [[] for _ in range(HG)]
            for h in range(HC):
                w2_sched[8 + (h * (HG - 8)) // HC].append(h)

            for g in range(HG):
                if g == 0:
                    w1g, w3g = w1g0, w3g0
                else:
                    w1g = wpool.tile([P, DC * H_SPAN], bf16, tag="w1g", name="w1g")
                    w3g = wpool.tile([P, DC * H_SPAN], bf16, tag="w3g", name="w3g")
                    nc.sync.dma_start(out=w1g[:], in_=w1t[g * P : (g + 1) * P, :])
                    nc.scalar.dma_start(out=w3g[:], in_=w3t[g * P : (g + 1) * P, :])
                for h in w2_sched[g]:
                    t = w2pool.tile([P, D_SPAN], bf16, tag=f"w2_{h}", name=f"w2_{h}")
                    nc.sync.dma_start(out=t[:], in_=w2t[h * P : (h + 1) * P, :])
                    w2g0_tiles[h] = t
                for hl in range(HL):
                    hrow = g * HL + hl
                    htile = hpool.tile(
                        [P, n_pad - chunks[ND][0] if ND < NCH else P],
                        bf16, tag="ht", name="ht",
                    )
                    pgs = [
                        pspool.tile(
                            [P, sz], f32, tag=f"pg{j}",
                            bufs=(2 if j == 0 else 1), name=f"pg{j}",
                        )
                        for j, (n0, sz) in enumerate(chunks)
                    ]
                    pvs = [
                        pspool.tile([P, sz], f32, tag=f"pv{j}", name=f"pv{j}")
                        for j, (n0, sz) in enumerate(chunks)
                    ]
                    for c in range(DC):
                        lhs = w1g[:, c * H_SPAN + hl * P : c * H_SPAN + hl * P + P]
                        for j, (n0, sz) in enumerate(chunks):
                            nc.tensor.matmul(
                                pgs[j][:],
                                lhs,
                                xslice(c, n0, sz),
                                start=(c == 0),
                                stop=(c == DC - 1),
                            )
                    for c in range(DC):
                        lhs = w3g[:, c * H_SPAN + hl * P : c * H_SPAN + hl * P + P]
                        for j, (n0, sz) in enumerate(chunks):
                            nc.tensor.matmul(
                                pvs[j][:],
                                lhs,
                                xslice(c, n0, sz),
                                start=(c == 0),
                                stop=(c == DC - 1),
                            )
                    for j, (n0, sz) in enumerate(chunks):
                        sg_t = spool.tile([P, sz], f32, tag="sg", name="sg_t")
                        nc.scalar.activation(sg_t[:], pgs[j][:], Sigmoid)
                        gv_t = spool.tile([P, sz], f32, tag="gv", name="gv_t")
                        nc.vector.tensor_tensor(gv_t[:], sg_t[:], pgs[j][:], op=mult)
                        if j < ND:
                            # write the strip slice directly in SBUF
                            nc.vector.tensor_tensor(
                                hstrips[j][:, hrow * sz : (hrow + 1) * sz],
                                gv_t[:], pvs[j][:], op=mult,
                            )
                        else:
                            off = n0 - chunks[ND][0]
                            nc.vector.tensor_tensor(
                                htile[:, off : off + sz], gv_t[:], pvs[j][:],
                                op=mult,
                            )
                            nc.gpsimd.dma_start(
                                out=hbs[j][hrow * P : (hrow + 1) * P, :],
                                in_=htile[:, off : off + sz],
                            )

        # ---- phase 2: y^T = W2^T H^T
        with ExitStack() as p2:
            hinpool = p2.enter_context(tc.tile_pool(name="hin", bufs=1))
            ps2 = p2.enter_context(tc.tile_pool(name="ps2", bufs=2, space="PSUM"))
            ypool = p2.enter_context(tc.tile_pool(name="yst", bufs=4))

            # load the DRAM-staged strips (ready well before their chunk runs)
            for j in range(ND, NCH):
                sz = chunks[j][1]
                hs = hinpool.tile([P, HC * sz], bf16, tag=f"hs{j}", name=f"hs{j}")
                for h in range(HC):
                    nc.gpsimd.dma_start(
                        out=hs[:, h * sz : (h + 1) * sz],
                        in_=hbs[j][h * P : (h + 1) * P, :],
                    )
                hstrips[j] = hs

            for dg in range(DG):
                d0 = dg * D_SPAN
                if dg == 0:
                    w2g = w2g0_tiles
                else:
                    w2g = []
                    for h in range(HC):
                        t = w2pool.tile(
                            [P, D_SPAN], bf16, tag=f"w2_{h}", name=f"w2_{h}"
                        )
                        eng = nc.sync if h % 2 == 0 else nc.scalar
                        eng.dma_start(
                            out=t[:],
                            in_=w2t[(dg * HC + h) * P : (dg * HC + h + 1) * P, :],
                        )
                        w2g.append(t)
                for j, (n0, sz) in enumerate(chunks):
                    hstrip = hstrips[j]
                    ps = [
                        ps2.tile([P, sz], f32, tag=f"yp{q}", name=f"yp{q}")
                        for q in range(DT_SWEEP)
                    ]
                    last_group = dg == DG - 1 and j == NCH - 1

                    def emit_copy(q):
                        yst = ypool.tile([P, sz], bf16, tag="yst", name="yst")
                        if q % 2 == 0:
                            nc.scalar.copy(yst[:], ps[q][:])
                        else:
                            nc.vector.tensor_copy(yst[:], ps[q][:])
                        nc.gpsimd.dma_start(
                            out=yts[j][d0 + q * P : d0 + (q + 1) * P, :],
                            in_=yst[:],
                        )

                    if last_group:
                        # q-outer so each psum finishes early and its copy +
                        # store overlap the remaining matmuls (short drain)
                        for q in range(DT_SWEEP):
                            for h in range(HC):
                                nc.tensor.matmul(
                                    ps[q][:],
                                    w2g[h][:, q * P : (q + 1) * P],
                                    hstrip[:, h * sz : (h + 1) * sz],
                                    start=(h == 0),
                                    stop=(h == HC - 1),
                                )
                            emit_copy(q)
                    else:
                        for h in range(HC):
                            for q in range(DT_SWEEP):
                                nc.tensor.matmul(
                                    ps[q][:],
                                    w2g[h][:, q * P : (q + 1) * P],
                                    hstrip[:, h * sz : (h + 1) * sz],
                                    start=(h == 0),
                                    stop=(h == HC - 1),
                                )
                        for q in range(DT_SWEEP):
                            emit_copy(q)

    nc.compile()
    return nc


def _get_program(n_pad: int):
    if n_pad not in _prog_cache:
        _prog_cache[n_pad] = _build_program(D_MODEL, HIDDEN, n_pad)
    return _prog_cache[n_pad]


def _route(x2d: np.ndarray, Wg: np.ndarray):
    """Host gate: float64 softmax + top-2 + renormalize."""
    logits = x2d.astype(np.float64) @ Wg.astype(np.float64)
    logits -= logits.max(axis=-1, keepdims=True)
    e = np.exp(logits)
    p = e / e.sum(axis=-1, keepdims=True)
    top = np.argsort(-p, axis=-1, kind="stable")[:, :TOP_K]
    w = np.take_along_axis(p, top, axis=-1)
    w = w / w.sum(axis=-1, keepdims=True)
    return top, w.astype(np.float32)


def _retile_w13(w: np.ndarray) -> np.ndarray:
    """[d_model, hidden] -> [HG*P, DC*H_SPAN] per-group contiguous layout."""
    DC = D_MODEL // P
    HG = HIDDEN // H_SPAN
    return np.ascontiguousarray(
        w.reshape(DC, P, HG, H_SPAN).transpose(2, 1, 0, 3).reshape(HG * P, DC * H_SPAN)
    )


def _retile_w2(w: np.ndarray) -> np.ndarray:
    """[hidden, d_model] -> [DG*HC*P, D_SPAN] per-tile contiguous layout."""
    HC = HIDDEN // P
    DG = D_MODEL // D_SPAN
    return np.ascontiguousarray(
        w.reshape(HC, P, DG, D_SPAN).transpose(2, 0, 1, 3).reshape(DG * HC * P, D_SPAN)
    )


def _prepare(inputs: dict):
    x = np.asarray(inputs["x"], dtype=np.float32)
    Wg = np.asarray(inputs["Wg"], dtype=np.float32)
    W1 = np.asarray(inputs["W1"], dtype=np.float32)
    W3 = np.asarray(inputs["W3"], dtype=np.float32)
    W2 = np.asarray(inputs["W2"], dtype=np.float32)

    b, s, d = x.shape
    T = b * s
    x2d = np.ascontiguousarray(x.reshape(T, d))

    top, wts = _route(x2d, Wg)

    tok_lists = []
    wt_lists = []
    for e in range(N_EXPERTS):
        mask = top == e  # [T, K]
        toks = np.where(mask.any(axis=-1))[0]
        we = wts[toks][mask[toks]]  # one weight per selected token
        tok_lists.append(toks)
        wt_lists.append(we.astype(np.float32))

    max_count = max(len(t) for t in tok_lists)
    n_pad = max(((max_count + 3) // 4) * 4, 24)

    nc = _get_program(n_pad)

    W1bf = W1.astype(ml_dtypes.bfloat16)
    W3bf = W3.astype(ml_dtypes.bfloat16)
    W2bf = W2.astype(ml_dtypes.bfloat16)
    x2dbf = x2d.astype(ml_dtypes.bfloat16)
    in_maps = []
    for e in range(N_EXPERTS):
        toks = tok_lists[e]
        xTe = np.zeros((d, n_pad), dtype=ml_dtypes.bfloat16)
        xTe[:, : len(toks)] = x2dbf[toks].T
        in_maps.append(
            {
                "xT": xTe,
                "w1t": _retile_w13(W1bf[e]),
                "w3t": _retile_w13(W3bf[e]),
                "w2t": _retile_w2(W2bf[e]),
            }
        )

    return nc, in_maps, tok_lists, wt_lists, (b, s, d)


def _combine(results, tok_lists, wt_lists, shape):
    b, s, d = shape
    out2d = np.zeros((b * s, d), dtype=np.float32)
    for e in range(N_EXPERTS):
        toks = tok_lists[e]
        yTe = np.concatenate(
            [np.asarray(results[e][f"yt{j}"]) for j in range(len(results[e]))
             if f"yt{j}" in results[e]],
            axis=1,
        ).astype(np.float32)
        ye = yTe[:, : len(toks)].T  # [n_e, d]
        out2d[toks] += wt_lists[e][:, None] * ye
    return out2d.reshape(b, s, d)


def _ensure_trace_hooks():
    """If BASS_TRACE is set, run_bass_kernel_spmd imports antenv.axon_hooks,
    which some images lack. Provide the standard shim (ctypes into the axon
    .so) when missing, and make the artifact upload failure-tolerant."""
    import sys

    try:
        import antenv.axon_hooks  # noqa: F401
        return
    except ImportError:
        pass
    import contextlib
    import ctypes
    import types

    so_path = "/opt/axon/libaxon_pjrt.so"
    hook = None
    try:
        lib = ctypes.CDLL(so_path)
        lib.axon_start_nrt_profile.argtypes = [
            ctypes.POINTER(ctypes.c_int64),
            ctypes.c_size_t,
        ]
        lib.axon_start_nrt_profile.restype = ctypes.c_int64
        lib.axon_stop_nrt_profile.argtypes = [ctypes.c_char_p]
        lib.axon_stop_nrt_profile.restype = ctypes.c_int64

        @contextlib.contextmanager
        def _hook(output_dir, device_ids):
            import jax

            jax.devices()
            if device_ids:
                ids = (ctypes.c_int64 * len(device_ids))(*device_ids)
                rc = lib.axon_start_nrt_profile(ids, len(device_ids))
            else:
                rc = lib.axon_start_nrt_profile(None, 0)
            if rc != 0:
                raise RuntimeError(f"axon_start_nrt_profile rc={rc}")
            try:
                yield
            finally:
                lib.axon_stop_nrt_profile(str(output_dir).encode())

        hook = _hook
    except Exception:
        hook = None

    mod = types.ModuleType("antenv.axon_hooks")
    state = {"hook": hook}
    mod.get_axon_ntff_profile_hook = lambda: state["hook"]
    mod.set_axon_ntff_profile_hook = lambda h: state.update(hook=h)
    sys.modules["antenv.axon_hooks"] = mod
    try:
        import antenv

        antenv.axon_hooks = mod
    except ImportError:
        pass

    import concourse.bass_utils as bu

    orig_upload = bu.upload_artifacts

    def _safe_upload(tmpdir):
        try:
            return orig_upload(tmpdir)
        except Exception:
            return f"local://{tmpdir}"

    bu.upload_artifacts = _safe_upload


def kernel(**inputs) -> np.ndarray:
    from concourse.bass_utils import run_bass_kernel_spmd

    _ensure_trace_hooks()
    nc, in_maps, tok_lists, wt_lists, shape = _prepare(inputs)
    res = run_bass_kernel_spmd(nc, in_maps, core_ids=list(range(N_CORES)))
    return _combine(res.results, tok_lists, wt_lists, shape)
